# revision 7
# baseline (speedup 1.0000x reference)
"""DGCNN encoder Trainium2 kernel v3 (batch-parallel over 8 NeuronCores).

Per core, one sample x (3, 2048). EdgeConv collapses algebraically:
with f = cat(nbr-ctr, ctr), conv+BN+ReLU+max over k becomes
  x_out[o,n] = relu( max_{m in knn(n)} P[o,m] + Q[o,n] )
  P = (s*W_nbr) x,  Q = (s*(W_ctr-W_nbr)) x + (s*(b-mu)+beta),  s >= 0.
KNN scores score[n,m] = 2<x_n,x_m> - |x_m|^2 (row-constant term dropped).
Exact fp32 top-20 per row (DVE max8/max_index/match_replace; the selection
is chaotically sensitive — 16-bit rounding breaks the 2e-2 gate).

v3 design:
  - P kept in SBUF channel-major [O, N]; neighbor rows fetched with
    ap_gather (Q7 SBUF-local free-axis gather, indices shared across
    channels) — no DRAM round trip, no PE transposes, ~0 DMA traffic.
  - gather indices wrapped to the Q7 [16 x NI/16] layout via a DRAM hop
    (contiguous dump + strided re-read) on the idle sync engine.
  - scores for layers 1-2 via a single matmul per chunk: x carries an
    appended ones row, aug carries [2x; -|x|^2].
  - neighbor max: one strided tensor_reduce; +Q and ReLU+bias fused as
    one DVE add + one scalar activation (bias = folded BN column).
  - software pipelining: tile t's fold/emit is issued after tile t+1's
    top-k so the DVE never stalls on the gather latency.
"""
import numpy as np

import concourse.bacc as bacc
import concourse.bass as bass
import concourse.mybir as mybir
from concourse.tile import TileContext
from concourse.bass_utils import run_bass_kernel_spmd
from concourse import library_config

F32 = mybir.dt.float32
U16 = mybir.dt.uint16
I16 = mybir.dt.int16
AX = mybir.AluOpType
AF = mybir.ActivationFunctionType

N = 2048
K = 20
NT = N // 128
NI = 128 * K          # flat gather idxs per tile
NW = NI // 16         # 160 wrapped i16 slots
EPS = 1e-5
NEG = -1e30

LAYERS = [(3, 64), (64, 128), (128, 256)]

_cache = {}


def _fold_host(inputs):
    out = {}
    for li, (C, O) in enumerate(LAYERS, start=1):
        w = inputs[f'w{li}']; b = inputs[f'b{li}']; g = inputs[f'g{li}']
        be = inputs[f'be{li}']; m = inputs[f'm{li}']; v = inputs[f'v{li}']
        s = g / np.sqrt(v + EPS)
        A = (s[:, None] * w[:, :C]).astype(np.float32)
        B = (s[:, None] * (w[:, C:] - w[:, :C])).astype(np.float32)
        c = (s * (b - m) + be).astype(np.float32)
        nob = max(1, O // 128)
        out[f'AT{li}'] = np.ascontiguousarray(A.T)
        out[f'BT{li}'] = np.ascontiguousarray(B.T)
        cbm = c.reshape(nob, -1).T  # [O/nob, nob]
        cbp = np.zeros((128, nob), dtype=np.float32)
        cbp[:cbm.shape[0]] = cbm
        out[f'cb{li}'] = cbp
    so = inputs['go'] / np.sqrt(inputs['vo'] + EPS)
    Ao = (so[:, None] * inputs['wo']).astype(np.float32)
    co = (so * (inputs['bo'] - inputs['mo']) + inputs['beo']).astype(np.float32)
    AoT = np.ascontiguousarray(Ao.T)
    out['AoT1'] = np.ascontiguousarray(AoT[0:64])
    out['AoT2'] = np.ascontiguousarray(AoT[64:192])
    out['AoT3a'] = np.ascontiguousarray(AoT[192:320])
    out['AoT3b'] = np.ascontiguousarray(AoT[320:448])
    out['co'] = np.ascontiguousarray(co.reshape(4, 128).T)
    return out


class _Builder:
    def __init__(self):
        self.nc = bacc.Bacc(None, target_bir_lowering=False, debug=False)
        self.d = {}

    def inp(self, name, shape, dtype=F32):
        self.d[name] = self.nc.dram_tensor(name, shape, dtype, kind="ExternalInput")

    def edge_layer(self, xt, li, C, O, has_ones):
        """xt: [C(+1), N] tile; rows 0:C are x, row C is ones iff has_ones.
        Returns x_next tiles; x_next[0] has an appended ones row iff O == 64."""
        nc, pp, wp = self.nc, self.pp, self.wp
        mmps, auxps = self.mmps, self.auxps
        ones = self.ones
        nob = max(1, O // 128)

        ATs = pp.tile([C, O], F32, name=f"ATs{li}", tag=f"ATs{li}")
        BTs = pp.tile([C, O], F32, name=f"BTs{li}", tag=f"BTs{li}")
        cbc = pp.tile([128, nob], F32, name=f"cbc{li}", tag=f"cbc{li}")
        nc.sync.dma_start(ATs[:], self.d[f'AT{li}'][:])
        nc.sync.dma_start(BTs[:], self.d[f'BT{li}'][:])
        nc.sync.dma_start(cbc[:], self.d[f'cb{li}'][:])

        x = xt[0:C, :]

        # aug rows 0:C = 2x; row C = -|x|^2 when the ones-trick is in play
        crow = C + 1 if has_ones else C
        aug = pp.tile([crow, N], F32, name=f"aug{li}", tag=f"aug{li}")
        sq = wp.tile([C, N], F32, name=f"sq{li}", tag="sq")
        nc.scalar.activation(out=aug[0:C, :], in_=x, func=AF.Copy, scale=2.0)
        nc.scalar.activation(out=sq[:], in_=x, func=AF.Square)
        # engine ops must start at partition 0/32/64/96 — stage negxx in a
        # [1, N] tile and DMA it into aug row C when C isn't a legal start
        direct = has_ones and (C % 32 == 0)
        if direct:
            negxx = aug[C:C+1, :]
        else:
            negxx_t = pp.tile([1, N], F32, name=f"negxx{li}", tag=f"negxx{li}")
            negxx = negxx_t[:]
        for ch in range(4):
            xx_ps = auxps.tile([1, 512], F32, name=f"xxps{li}_{ch}", tag="xx_ps",
                               space="PSUM")
            nc.tensor.matmul(out=xx_ps[:], lhsT=ones[0:C, 0:1],
                             rhs=sq[:, ch*512:(ch+1)*512], start=True, stop=True)
            nc.scalar.activation(out=negxx[0:1, ch*512:(ch+1)*512], in_=xx_ps[:],
                                 func=AF.Copy, scale=-1.0)
        if has_ones and not direct:
            nc.sync.dma_start(aug[C:C+1, :], negxx)

        # P = A x, channel-major [O, N] in SBUF (ap_gather source)
        P_sb = [pp.tile([min(128, O - i*128), N], F32, name=f"P{li}_{i}",
                        tag=f"P{li}_{i}") for i in range(nob)]
        for t in range(NT):
            tsl = slice(t * 128, (t + 1) * 128)
            for ob in range(nob):
                obs = slice(ob * 128, min((ob + 1) * 128, O))
                w = obs.stop - obs.start
                pt_ps = auxps.tile([128, 128], F32, name=f"ptps{li}_{t}_{ob}",
                                   tag="pt_ps", space="PSUM")
                nc.tensor.matmul(out=pt_ps[0:w, :], lhsT=ATs[:, obs], rhs=x[:, tsl],
                                 start=True, stop=True)
                nc.scalar.copy(out=P_sb[ob][0:w, tsl], in_=pt_ps[0:w, :])

        # x_next tiles; first block gets a ones row if next layer uses the trick
        next_ones = (O == 64)
        x_next = [pp.tile([min(128, O - i*128) + (1 if (next_ones and i == 0) else 0),
                           N], F32, name=f"xn{li}_{i}", tag=f"xn{li}_{i}")
                  for i in range(nob)]
        if next_ones:
            nc.vector.memset(x_next[0][O:O+1, :], 1.0)

        # --- software-pipelined tile loop -------------------------------
        stash = {}

        def front(t):
            tsl = slice(t * 128, (t + 1) * 128)
            scoreS = wp.tile([128, N], F32, name=f"sc{li}_{t}", tag="scoreS")
            for ch in range(4):
                csl = slice(ch * 512, (ch + 1) * 512)
                sc_ps = mmps.tile([128, 512], F32, name=f"scps{li}_{t}_{ch}",
                                  tag="mm_ps", space="PSUM")
                if has_ones:
                    nc.tensor.matmul(out=sc_ps[:], lhsT=xt[:, tsl],
                                     rhs=aug[:, csl], start=True, stop=True)
                else:
                    nc.tensor.matmul(out=sc_ps[:], lhsT=x[:, tsl],
                                     rhs=aug[0:C, csl], start=True, stop=False)
                    nc.tensor.matmul(out=sc_ps[:], lhsT=ones[0:1, 0:128],
                                     rhs=negxx[0:1, csl], start=False, stop=True)
                nc.scalar.copy(out=scoreS[:, csl], in_=sc_ps[:])

            mx = wp.tile([128, 24], F32, name=f"mx{li}_{t}", tag="mx")
            ixu = wp.tile([128, 24], U16, name=f"ixu{li}_{t}", tag="ixu")
            for r in range(3):
                rsl = slice(r * 8, (r + 1) * 8)
                nc.vector.max(out=mx[:, rsl], in_=scoreS[:])
                nc.vector.max_index(out=ixu[:, rsl], in_max=mx[:, rsl],
                                    in_values=scoreS[:])
                if r < 2:
                    nc.vector.match_replace(out=scoreS[:], in_to_replace=mx[:, rsl],
                                            in_values=scoreS[:], imm_value=NEG)

            # wrap indices (flat i = j*128 + p) into the Q7 [16, NW] layout
            T1d = self.dpool.tile([128, K], U16, name=f"t1d{li}_{t}", tag="t1d")
            nc.sync.dma_start(T1d[:], ixu[:, 0:K])
            W = wp.tile([128, NW], I16, name=f"W{li}_{t}", tag="W")
            nc.sync.dma_start(
                W[0:16, :],
                T1d[:].bitcast(I16).rearrange("(a b) j -> b j a", a=8, b=16))
            nc.sync.dma_start(W[16:32, :], W[0:16, :])
            nc.sync.dma_start(W[32:64, :], W[0:32, :])
            nc.sync.dma_start(W[64:128, :], W[0:64, :])

            gs = []
            for ob in range(nob):
                w = P_sb[ob].shape[0]
                g = wp.tile([w, NI], F32, name=f"g{li}_{t}_{ob}", tag=f"gath{ob}")
                nc.gpsimd.ap_gather(out_ap=g[:], in_ap=P_sb[ob][:],
                                    idxs_ap=W[0:w, :], channels=w,
                                    num_elems=N, d=1, num_idxs=NI)
                gs.append(g)
            stash[t] = gs

        def back(t):
            tsl = slice(t * 128, (t + 1) * 128)
            gs = stash.pop(t)
            for ob in range(nob):
                obs = slice(ob * 128, min((ob + 1) * 128, O))
                w = obs.stop - obs.start
                qt_ps = auxps.tile([128, 128], F32, name=f"qt{li}_{t}_{ob}",
                                   tag="qt_ps", space="PSUM")
                nc.tensor.matmul(out=qt_ps[0:w, :], lhsT=BTs[:, obs],
                                 rhs=x[:, tsl], start=True, stop=True)
                fsum = wp.tile([w, 128], F32, name=f"fs{li}_{t}_{ob}", tag="fsum")
                # neighbor max over the 20 ranks (j stride 128), + Q
                nc.vector.tensor_reduce(
                    out=fsum[:],
                    in_=gs[ob][:].rearrange("c (j q) -> c q j", j=K, q=128),
                    axis=mybir.AxisListType.X, op=AX.max)
                nc.vector.tensor_tensor(out=fsum[:], in0=fsum[:],
                                        in1=qt_ps[0:w, :], op=AX.add)
                nc.scalar.activation(out=x_next[ob][0:w, tsl], in_=fsum[:],
                                     func=AF.Relu, bias=cbc[0:w, ob:ob+1],
                                     scale=1.0)

        for t in range(NT):
            front(t)
            if t >= 1:
                back(t - 1)
        back(NT - 1)
        return x_next

    def build(self):
        nc = self.nc
        self.inp('x', [3, N])
        for li, (C, O) in enumerate(LAYERS, start=1):
            nob = max(1, O // 128)
            self.inp(f'AT{li}', [C, O]); self.inp(f'BT{li}', [C, O])
            self.inp(f'cb{li}', [128, nob])
        self.inp('AoT1', [64, 512]); self.inp('AoT2', [128, 512])
        self.inp('AoT3a', [128, 512]); self.inp('AoT3b', [128, 512])
        self.inp('co', [128, 4])
        out_d = nc.dram_tensor('out', [512], F32, kind="ExternalOutput")

        with TileContext(nc) as tc:
            with (
                tc.tile_pool(name="pp", bufs=1) as pp,
                tc.tile_pool(name="wp", bufs=2) as wp,
                tc.tile_pool(name="mmps", bufs=2, space="PSUM") as mmps,
                tc.tile_pool(name="auxps", bufs=2, space="PSUM") as auxps,
                tc.tile_pool(name="dram", bufs=2, space="DRAM") as dpool,
            ):
                self.pp, self.wp = pp, wp
                self.mmps, self.auxps, self.dpool = mmps, auxps, dpool
                nc.gpsimd.load_library(library_config.ap_gather)
                ones = pp.tile([128, 128], F32, name="ones", tag="ones")
                nc.vector.memset(ones[:], 1.0)
                self.ones = ones
                x0 = pp.tile([4, N], F32, name="x0", tag="x0")
                nc.vector.memset(x0[:], 1.0)
                nc.sync.dma_start(x0[0:3, :], self.d['x'][:])

                x1 = self.edge_layer(x0, 1, 3, 64, True)[0]     # [65, N]
                x2 = self.edge_layer(x1, 2, 64, 128, True)[0]   # [128, N]
                x3a, x3b = self.edge_layer(x2, 3, 128, 256, False)

                specs = [('AoT1', x1, 64), ('AoT2', x2, 128),
                         ('AoT3a', x3a, 128), ('AoT3b', x3b, 128)]
                lhs_s = []
                for i, (nm, _, kk) in enumerate(specs):
                    ls = pp.tile([kk, 512], F32, name=f"Ao{i}", tag=f"Ao{i}")
                    nc.sync.dma_start(ls[:], self.d[nm][:])
                    lhs_s.append(ls)
                cos = pp.tile([128, 4], F32, name="cos", tag="cos")
                nc.sync.dma_start(cos[:], self.d['co'][:])

                for mc in range(4):
                    msl = slice(mc * 128, (mc + 1) * 128)
                    acc = wp.tile([128, 4], F32, name=f"acc{mc}", tag="acc")
                    red = wp.tile([128, 1], F32, name=f"red{mc}", tag="red")
                    for nchk in range(4):
                        nsl = slice(nchk * 512, (nchk + 1) * 512)
                        y_ps = mmps.tile([128, 512], F32, name=f"y{mc}_{nchk}",
                                         tag="mm_ps", space="PSUM")
                        for ki, (_, xs, kk) in enumerate(specs):
                            nc.tensor.matmul(out=y_ps[:], lhsT=lhs_s[ki][:, msl],
                                             rhs=xs[0:kk, nsl],
                                             start=(ki == 0), stop=(ki == 3))
                        y_sb = wp.tile([128, 512], F32, name=f"ysb{mc}_{nchk}",
                                       tag="y_sb")
                        nc.scalar.activation(out=y_sb[:], in_=y_ps[:], func=AF.Relu,
                                             bias=cos[:, mc:mc+1], scale=1.0)
                        nc.vector.tensor_reduce(out=acc[:, nchk:nchk+1], in_=y_sb[:],
                                                axis=mybir.AxisListType.X, op=AX.max)
                    nc.vector.tensor_reduce(out=red[:], in_=acc[:],
                                            axis=mybir.AxisListType.X, op=AX.max)
                    nc.sync.dma_start(out_d[msl], red[:])
        nc.compile()
        return nc


def build_kernel():
    return _Builder().build()


def kernel(**inputs):
    if 'nc' not in _cache:
        _cache['nc'] = build_kernel()
    nc = _cache['nc']
    folded = _fold_host(inputs)
    xs = np.asarray(inputs['x'], dtype=np.float32)
    in_maps = [{**folded, 'x': np.ascontiguousarray(xs[b])} for b in range(8)]
    res = run_bass_kernel_spmd(nc, in_maps, core_ids=list(range(8)))
    return np.stack([res.results[b]['out'] for b in range(8)]).astype(np.float32)


# revision 13
# speedup vs baseline: 2.6944x; 2.6944x over previous
"""DGCNN encoder Trainium2 kernel v4 (batch-parallel over 8 NeuronCores).

Per core, one sample x (3, 2048). EdgeConv collapses algebraically:
with f = cat(nbr-ctr, ctr), conv+BN+ReLU+max over k becomes
  x_out[o,n] = relu( max_{m in knn(n)} P[o,m] + Q[o,n] )
  P = (s*W_nbr) x,  Q = (s*(W_ctr-W_nbr)) x + (s*(b-mu)+beta),  s >= 0.
KNN scores score[n,m] = 2<x_n,x_m> - |x_m|^2 (row-constant term dropped).
Exact fp32 top-20 per row (DVE max8/max_index/match_replace; the selection
is chaotically sensitive — 16-bit rounding breaks the 2e-2 gate).

v4 design:
  - neighbor rows of P^T (DRAM [N, O]) fetched with dma_gather: one Q7
    SWDGE instruction per top-k round (1024/1024/512 idxs) instead of 20
    indirect DMAs; idx wrap layout built by the idle sync engine via a
    DRAM hop, per round, so gathers overlap the same tile's later rounds.
  - scores for layers 1-2 via a single matmul per chunk: x carries an
    appended ones row, aug carries [2x; -|x|^2].
  - neighbor max: one strided tensor_reduce; Q^T + transpose(gmax)
    accumulate in one PSUM bank; ReLU+bias via scalar ACT.
  - software pipelining: tile t's fold is emitted after tile t+1's top-k
    so the DVE never stalls on gather latency.
"""
import numpy as np

import concourse.bacc as bacc
import concourse.bass as bass
import concourse.mybir as mybir
from concourse.tile import TileContext
from concourse.bass_utils import run_bass_kernel_spmd
from concourse import library_config

F32 = mybir.dt.float32
U16 = mybir.dt.uint16
I16 = mybir.dt.int16
AX = mybir.AluOpType
AF = mybir.ActivationFunctionType

N = 2048
K = 20
NT = N // 128
NI = 128 * K          # flat gather idxs per tile
NW = NI // 16         # 160 wrapped i16 slots
EPS = 1e-5
NEG = -1e30

LAYERS = [(3, 64), (64, 128), (128, 256)]

_cache = {}


def _fold_host(inputs):
    out = {}
    for li, (C, O) in enumerate(LAYERS, start=1):
        w = inputs[f'w{li}']; b = inputs[f'b{li}']; g = inputs[f'g{li}']
        be = inputs[f'be{li}']; m = inputs[f'm{li}']; v = inputs[f'v{li}']
        s = g / np.sqrt(v + EPS)
        A = (s[:, None] * w[:, :C]).astype(np.float32)
        B = (s[:, None] * (w[:, C:] - w[:, :C])).astype(np.float32)
        c = (s * (b - m) + be).astype(np.float32)
        nob = max(1, O // 128)
        out[f'AT{li}'] = np.ascontiguousarray(A.T)
        out[f'BT{li}'] = np.ascontiguousarray(B.T)
        cbm = c.reshape(nob, -1).T  # [O/nob, nob]
        cbp = np.zeros((128, nob), dtype=np.float32)
        cbp[:cbm.shape[0]] = cbm
        out[f'cb{li}'] = cbp
    so = inputs['go'] / np.sqrt(inputs['vo'] + EPS)
    Ao = (so[:, None] * inputs['wo']).astype(np.float32)
    co = (so * (inputs['bo'] - inputs['mo']) + inputs['beo']).astype(np.float32)
    AoT = np.ascontiguousarray(Ao.T)
    out['AoT1'] = np.ascontiguousarray(AoT[0:64])
    out['AoT2'] = np.ascontiguousarray(AoT[64:192])
    out['AoT3a'] = np.ascontiguousarray(AoT[192:320])
    out['AoT3b'] = np.ascontiguousarray(AoT[320:448])
    out['co'] = np.ascontiguousarray(co.reshape(4, 128).T)
    out['identity'] = np.eye(128, dtype=np.float32)
    return out


class _Builder:
    def __init__(self):
        self.nc = bacc.Bacc(None, target_bir_lowering=False, debug=False)
        self.d = {}

    def inp(self, name, shape, dtype=F32):
        self.d[name] = self.nc.dram_tensor(name, shape, dtype, kind="ExternalInput")

    def edge_layer(self, xt, li, C, O, has_ones):
        """xt: [C(+1), N] tile; rows 0:C are x, row C is ones iff has_ones.
        Returns x_next tiles; x_next[0] has an appended ones row iff O == 64.
        Layer-local big tiles (aug, sq, negxx) live in a per-layer pool."""
        nc, pp, wp = self.nc, self.pp, self.wp
        with self.tc.tile_pool(name=f"lp{li}", bufs=1) as lp:
            return self._edge_layer_body(lp, xt, li, C, O, has_ones)

    def _edge_layer_body(self, lp, xt, li, C, O, has_ones):
        nc, pp, wp = self.nc, self.pp, self.wp
        mmps, auxps = self.mmps, self.auxps
        ones = self.ones
        nob = max(1, O // 128)

        ATs = pp.tile([C, O], F32, name=f"ATs{li}", tag=f"ATs{li}")
        BTs = pp.tile([C, O], F32, name=f"BTs{li}", tag=f"BTs{li}")
        cbc = pp.tile([128, nob], F32, name=f"cbc{li}", tag=f"cbc{li}")
        nc.sync.dma_start(ATs[:], self.d[f'AT{li}'][:])
        nc.sync.dma_start(BTs[:], self.d[f'BT{li}'][:])
        nc.sync.dma_start(cbc[:], self.d[f'cb{li}'][:])

        x = xt[0:C, :]

        # aug rows 0:C = 2x; row C = -|x|^2 when the ones-trick is in play
        crow = C + 1 if has_ones else C
        aug = lp.tile([crow, N], F32, name=f"aug{li}", tag=f"aug{li}")
        sq = lp.tile([C, N], F32, name=f"sq{li}", tag="sq")
        nc.scalar.activation(out=aug[0:C, :], in_=x, func=AF.Copy, scale=2.0)
        nc.scalar.activation(out=sq[:], in_=x, func=AF.Square)
        direct = has_ones and (C % 32 == 0)
        if direct:
            negxx = aug[C:C+1, :]
        else:
            negxx_t = lp.tile([1, N], F32, name=f"negxx{li}", tag=f"negxx{li}")
            negxx = negxx_t[:]
        for ch in range(4):
            xx_ps = auxps.tile([1, 512], F32, name=f"xxps{li}_{ch}", tag="xx_ps",
                               space="PSUM")
            nc.tensor.matmul(out=xx_ps[:], lhsT=ones[0:C, 0:1],
                             rhs=sq[:, ch*512:(ch+1)*512], start=True, stop=True)
            nc.scalar.activation(out=negxx[0:1, ch*512:(ch+1)*512], in_=xx_ps[:],
                                 func=AF.Copy, scale=-1.0)
        if has_ones and not direct:
            nc.sync.dma_start(aug[C:C+1, :], negxx)

        # P^T -> DRAM [N, O] for the row gathers
        PTd = self.dpool.tile([N, O], F32, name=f"PTd{li}", tag=f"PTd{li}")
        for t in range(NT):
            tsl = slice(t * 128, (t + 1) * 128)
            pt_ps = auxps.tile([128, O], F32, name=f"ptps{li}_{t}", tag="pt_ps",
                               space="PSUM")
            nc.tensor.matmul(out=pt_ps[:], lhsT=x[:, tsl], rhs=ATs[:],
                             start=True, stop=True)
            pt_sb = wp.tile([128, 256], F32, name=f"ptsb{li}_{t}", tag="pt_sb")
            nc.scalar.copy(out=pt_sb[:, 0:O], in_=pt_ps[:])
            nc.sync.dma_start(PTd[tsl, :], pt_sb[:, 0:O])

        # x_next tiles; first block gets a ones row if next layer uses the trick
        next_ones = (O == 64)
        x_next = [pp.tile([min(128, O - i*128) + (1 if (next_ones and i == 0) else 0),
                           N], F32, name=f"xn{li}_{i}", tag=f"xn{li}_{i}")
                  for i in range(nob)]
        if next_ones:
            nc.vector.memset(x_next[0][O:O+1, :], 1.0)

        # --- software-pipelined tile loop -------------------------------
        stash = {}
        RSPEC = [(0, 8, 0, 64, 1024), (8, 16, 64, 128, 1024),
                 (16, 20, 128, 160, 512)]

        def front(t):
            tsl = slice(t * 128, (t + 1) * 128)
            scoreS = wp.tile([128, N], F32, name=f"sc{li}_{t}", tag="scoreS")
            for ch in range(4):
                csl = slice(ch * 512, (ch + 1) * 512)
                sc_ps = mmps.tile([128, 512], F32, name=f"scps{li}_{t}_{ch}",
                                  tag="mm_ps", space="PSUM")
                if has_ones:
                    nc.tensor.matmul(out=sc_ps[:], lhsT=xt[:, tsl],
                                     rhs=aug[:, csl], start=True, stop=True)
                else:
                    nc.tensor.matmul(out=sc_ps[:], lhsT=x[:, tsl],
                                     rhs=aug[0:C, csl], start=True, stop=False)
                    nc.tensor.matmul(out=sc_ps[:], lhsT=ones[0:1, 0:128],
                                     rhs=negxx[0:1, csl], start=False, stop=True)
                nc.scalar.copy(out=scoreS[:, csl], in_=sc_ps[:])

            mx = wp.tile([128, 24], F32, name=f"mx{li}_{t}", tag="mx")
            ixu = wp.tile([128, 24], U16, name=f"ixu{li}_{t}", tag="ixu")
            T1d = self.dpool.tile([128, K], U16, name=f"t1d{li}_{t}", tag="t1d")
            W = wp.tile([128, NW], I16, name=f"W{li}_{t}", tag="W")
            g_base = wp.tile([128, K * 256], F32, name=f"g{li}_{t}", tag="gath")
            g = g_base[:, 0:K*O].rearrange("p (j o) -> p j o", j=K, o=O)
            for r, (j0, j1, s0, s1, ni) in enumerate(RSPEC):
                rsl = slice(r * 8, (r + 1) * 8)
                nc.vector.max(out=mx[:, rsl], in_=scoreS[:])
                nc.vector.max_index(out=ixu[:, rsl], in_max=mx[:, rsl],
                                    in_values=scoreS[:])
                if r < 2:
                    nc.vector.match_replace(out=scoreS[:], in_to_replace=mx[:, rsl],
                                            in_values=scoreS[:], imm_value=NEG)
                # dump this round's ranks to DRAM as soon as they exist
                jsl = slice(j0, j1)
                nc.sync.dma_start(T1d[:, jsl], ixu[:, jsl])
            # wrap all 20 ranks into the Q7 [16, NW] layout, then gather
            nc.sync.dma_start(
                W[0:16, :],
                T1d[:].bitcast(I16).rearrange("(a b) j -> b j a", a=8, b=16))
            nc.sync.dma_start(W[16:32, :], W[0:16, :])
            nc.sync.dma_start(W[32:64, :], W[0:32, :])
            nc.sync.dma_start(W[64:128, :], W[0:64, :])
            for r, (j0, j1, s0, s1, ni) in enumerate(RSPEC):
                nc.gpsimd.dma_gather(
                    out_ap=g[:, j0:j1, :], in_ap=PTd[:], idxs_ap=W[:, s0:s1],
                    num_idxs=ni, num_idxs_reg=ni, elem_size=O)
            stash[t] = g

        def back(t):
            tsl = slice(t * 128, (t + 1) * 128)
            g = stash.pop(t)
            gfold = wp.tile([128, 256], F32, name=f"gf{li}_{t}", tag="gfold")
            nc.vector.tensor_reduce(out=gfold[:, 0:O],
                                    in_=g.rearrange("p j o -> p o j"),
                                    axis=mybir.AxisListType.X, op=AX.max)
            for ob in range(nob):
                obs = slice(ob * 128, min((ob + 1) * 128, O))
                w = obs.stop - obs.start
                qt_ps = auxps.tile([128, 128], F32, name=f"qt{li}_{t}_{ob}",
                                   tag="qt_ps", space="PSUM")
                nc.tensor.matmul(out=qt_ps[0:w, :], lhsT=BTs[:, obs],
                                 rhs=x[:, tsl], start=True, stop=False)
                nc.tensor.matmul(out=qt_ps[0:w, :], lhsT=gfold[:, obs],
                                 rhs=self.ident[:], is_transpose=True,
                                 start=False, stop=True)
                nc.scalar.activation(out=x_next[ob][0:w, tsl], in_=qt_ps[0:w, :],
                                     func=AF.Relu, bias=cbc[0:w, ob:ob+1],
                                     scale=1.0)

        for t in range(NT):
            front(t)
            if t >= 2:
                back(t - 2)
        back(NT - 2)
        back(NT - 1)
        return x_next

    def build(self):
        nc = self.nc
        self.inp('x', [3, N])
        for li, (C, O) in enumerate(LAYERS, start=1):
            nob = max(1, O // 128)
            self.inp(f'AT{li}', [C, O]); self.inp(f'BT{li}', [C, O])
            self.inp(f'cb{li}', [128, nob])
        self.inp('AoT1', [64, 512]); self.inp('AoT2', [128, 512])
        self.inp('AoT3a', [128, 512]); self.inp('AoT3b', [128, 512])
        self.inp('co', [128, 4]); self.inp('identity', [128, 128])
        out_d = nc.dram_tensor('out', [512], F32, kind="ExternalOutput")

        with TileContext(nc) as tc:
            self.tc = tc
            with (
                tc.tile_pool(name="pp", bufs=1) as pp,
                tc.tile_pool(name="wp", bufs=3) as wp,
                tc.tile_pool(name="mmps", bufs=2, space="PSUM") as mmps,
                tc.tile_pool(name="auxps", bufs=2, space="PSUM") as auxps,
                tc.tile_pool(name="dram", bufs=3, space="DRAM") as dpool,
            ):
                self.pp, self.wp = pp, wp
                self.mmps, self.auxps, self.dpool = mmps, auxps, dpool
                nc.gpsimd.load_library(library_config.mlp)
                ones = pp.tile([128, 128], F32, name="ones", tag="ones")
                nc.vector.memset(ones[:], 1.0)
                self.ones = ones
                ident = pp.tile([128, 128], F32, name="identS", tag="identS")
                nc.sync.dma_start(ident[:], self.d['identity'][:])
                self.ident = ident
                x0 = pp.tile([4, N], F32, name="x0", tag="x0")
                nc.vector.memset(x0[:], 1.0)
                nc.sync.dma_start(x0[0:3, :], self.d['x'][:])

                x1 = self.edge_layer(x0, 1, 3, 64, True)[0]     # [65, N]
                x2 = self.edge_layer(x1, 2, 64, 128, True)[0]   # [128, N]
                x3a, x3b = self.edge_layer(x2, 3, 128, 256, False)

                specs = [('AoT1', x1, 64), ('AoT2', x2, 128),
                         ('AoT3a', x3a, 128), ('AoT3b', x3b, 128)]
                lhs_s = []
                for i, (nm, _, kk) in enumerate(specs):
                    ls = pp.tile([kk, 512], F32, name=f"Ao{i}", tag=f"Ao{i}")
                    nc.sync.dma_start(ls[:], self.d[nm][:])
                    lhs_s.append(ls)
                cos = pp.tile([128, 4], F32, name="cos", tag="cos")
                nc.sync.dma_start(cos[:], self.d['co'][:])

                for mc in range(4):
                    msl = slice(mc * 128, (mc + 1) * 128)
                    acc = wp.tile([128, 4], F32, name=f"acc{mc}", tag="acc")
                    red = wp.tile([128, 1], F32, name=f"red{mc}", tag="red")
                    for nchk in range(4):
                        nsl = slice(nchk * 512, (nchk + 1) * 512)
                        y_ps = mmps.tile([128, 512], F32, name=f"y{mc}_{nchk}",
                                         tag="mm_ps", space="PSUM")
                        for ki, (_, xs, kk) in enumerate(specs):
                            nc.tensor.matmul(out=y_ps[:], lhsT=lhs_s[ki][:, msl],
                                             rhs=xs[0:kk, nsl],
                                             start=(ki == 0), stop=(ki == 3))
                        y_sb = wp.tile([128, 512], F32, name=f"ysb{mc}_{nchk}",
                                       tag="y_sb")
                        nc.scalar.activation(out=y_sb[:], in_=y_ps[:], func=AF.Relu,
                                             bias=cos[:, mc:mc+1], scale=1.0)
                        nc.vector.tensor_reduce(out=acc[:, nchk:nchk+1], in_=y_sb[:],
                                                axis=mybir.AxisListType.X, op=AX.max)
                    nc.vector.tensor_reduce(out=red[:], in_=acc[:],
                                            axis=mybir.AxisListType.X, op=AX.max)
                    nc.sync.dma_start(out_d[msl], red[:])
        nc.compile()
        return nc


def build_kernel():
    return _Builder().build()


def kernel(**inputs):
    if 'nc' not in _cache:
        _cache['nc'] = build_kernel()
    nc = _cache['nc']
    folded = _fold_host(inputs)
    xs = np.asarray(inputs['x'], dtype=np.float32)
    in_maps = [{**folded, 'x': np.ascontiguousarray(xs[b])} for b in range(8)]
    res = run_bass_kernel_spmd(nc, in_maps, core_ids=list(range(8)))
    return np.stack([res.results[b]['out'] for b in range(8)]).astype(np.float32)
